# revision 1
# baseline (speedup 1.0000x reference)
# Order-2 CRF loss kernel for Trainium2 (Bass/Tile), 8-core data parallel.
#
# Math: the reference forward algorithm is, in linear domain, a pure matvec
# chain per batch row:
#     alpha_0[c] = exp(emits[b, 0, BOS*128 + c])
#     alpha_t = alpha_{t-1} @ E_t,   E_t = exp(em_t)  (em_t: [128 prev, 128 cur])
#     log_z_row = log(sum_c alpha_S[c])
# With N(0,1) emissions each step multiplies the magnitude by ~128*e^0.5, so we
# fold a constant shift DELTA = log(128)+0.5 into the exp bias
# (E'_t = exp(em_t - DELTA)); the chain then stays O(1) in magnitude (empirical
# drift < +-2 bits over 255 steps) and the host adds back
# DELTA * n_unmasked_steps at the end.  No renormalization on device.
#
# On device per core (2 batch rows): stream emissions HBM->SBUF in chunks,
# exp on ScalarE (bias=-DELTA), then per step a single TensorE matvec
# out[128,1] = E_t^T(stationary) @ alpha(moving) into PSUM and a VectorE copy
# back to SBUF.  Output per core: final alpha columns [128, 2].
#
# Host: gold-score gather, mask bookkeeping, final log/sum in float64.
# Masked steps (never present in the graded inputs, where mask is all ones)
# are handled exactly by overwriting that step's emissions with a
# "log-identity" pattern so the step multiplies alpha by I.

import numpy as np

import concourse.bass as bass
import concourse.tile as tile
from concourse import bacc, mybir
from concourse.bass_utils import run_bass_kernel_spmd

B, S, LO = 16, 256, 128
NL = LO * LO
N_CORES = 8
RPC = B // N_CORES  # rows per core = 2
DELTA = float(np.log(128.0) + 0.5)
CHUNK = 32  # scan steps per DMA chunk
MM_DTYPE = mybir.dt.bfloat16  # matvec operand dtype (exp output / alpha chain)

LAST_RESULTS = None  # BassKernelResults of the most recent run (for test.py)


def _build_program(repeats=1):
    """Build the per-core program.  `repeats` re-runs the whole streaming
    scan that many times inside one NEFF (used only for benchmarking: the
    difference between two repeat counts isolates kernel time from the
    per-dispatch overhead of the runtime)."""
    nc = bacc.Bacc("TRN2", target_bir_lowering=False, debug=False)
    emits_h = nc.dram_tensor(
        "emits", [RPC, S, NL], mybir.dt.float32, kind="ExternalInput"
    )
    alpha_out = nc.dram_tensor(
        "alpha_out", [LO, RPC], mybir.dt.float32, kind="ExternalOutput"
    )

    # [p, r, s, c] view of the emissions: partition = prev label.
    e_prsc = emits_h.rearrange("r s (p c) -> p r s c", p=LO)

    # chunk boundaries over scan steps t = 1..S-1
    starts = [1] + list(range(CHUNK, S, CHUNK))
    bounds = [(t0, min(t0 + CHUNK, S)) for t0 in starts]

    with tile.TileContext(nc) as tc:
        with (
            tc.tile_pool(name="raw", bufs=2) as raw_pool,
            tc.tile_pool(name="expo", bufs=2) as exp_pool,
            tc.tile_pool(name="alpha", bufs=4) as alpha_pool,
            tc.tile_pool(name="init", bufs=1) as init_pool,
            tc.tile_pool(name="psum", bufs=2, space="PSUM") as psum_pool,
        ):
            # per-partition bias constant for exp(x - DELTA)
            bias_t = init_pool.tile([LO, 1], mybir.dt.float32, name="bias_delta")
            nc.vector.memset(bias_t[:, :], -DELTA)

            # ---- init: alpha0 = exp(emits[r, 0, 0:128]) as a [128,1] column
            alpha_cur = []
            for r in range(RPC):
                la0 = init_pool.tile([LO, 1], mybir.dt.float32, name=f"la0_{r}")
                nc.sync.dma_start(
                    out=la0[:, :],
                    in_=emits_h[r, 0, 0:LO].rearrange("(p one) -> p one", one=1),
                )
                a0 = alpha_pool.tile(
                    [LO, 1], MM_DTYPE, tag=f"al{r}", name=f"alpha0_{r}"
                )
                nc.scalar.activation(
                    a0[:, :], la0[:, :], mybir.ActivationFunctionType.Exp
                )
                alpha_cur.append(a0)

            # ---- main chunked pipeline
            all_bounds = [(rep, t0, t1) for rep in range(repeats) for t0, t1 in bounds]
            for rep, t0, t1 in all_bounds:
                n = t1 - t0
                em_raw = raw_pool.tile(
                    [LO, RPC, n, LO], mybir.dt.float32, tag="raw", name="em_raw"
                )
                for r in range(RPC):
                    nc.sync.dma_start(
                        out=em_raw[:, r, :, :], in_=e_prsc[:, r, t0:t1, :]
                    )

                em_exp = exp_pool.tile(
                    [LO, RPC, n, LO], MM_DTYPE, tag="expo", name="em_exp"
                )
                for r in range(RPC):
                    for g0 in range(0, n, 8):
                        g1 = min(g0 + 8, n)
                        nc.scalar.activation(
                            em_exp[:, r, g0:g1, :],
                            em_raw[:, r, g0:g1, :],
                            mybir.ActivationFunctionType.Exp,
                            bias=bias_t[:, :],
                        )

                for t in range(t0, t1):
                    for r in range(RPC):
                        ps = psum_pool.tile(
                            [LO, 1], mybir.dt.float32, tag=f"ps{r}", name=f"ps_{r}"
                        )
                        nc.tensor.matmul(
                            ps[:, :],
                            em_exp[:, r, t - t0, :],
                            alpha_cur[r][:, :],
                            start=True,
                            stop=True,
                        )
                        # keep the final step's alpha in fp32 for the output DMA
                        last = rep == repeats - 1 and t == S - 1
                        a_dt = mybir.dt.float32 if last else MM_DTYPE
                        a_new = alpha_pool.tile(
                            [LO, 1], a_dt, tag=f"al{r}", name=f"alpha_{r}_{t}"
                        )
                        nc.vector.tensor_copy(a_new[:, :], ps[:, :])
                        alpha_cur[r] = a_new

            # ---- write out the final alpha columns
            for r in range(RPC):
                nc.sync.dma_start(
                    out=alpha_out[:, r : r + 1], in_=alpha_cur[r][:, :]
                )

    nc.compile()
    return nc


def _build_program_v2(repeats=1):
    """Two parallel chain segments per row + rank-1 junction stitching.

    Segment A: steps 1..MID-1 from alpha0.  Segment B: steps MID..S-1 from a
    vector of ones.  Because the positive transition matrices contract at
    ~1/sqrt(128) per step, alpha(S) is proportional to B's result, with the
    scalar recovered from k=JK extra steps of B applied to A's result:
        log Z = log sum(uB) + log sum(gA) - log sum(gW) + 255*DELTA
    where gA = (first JK steps of B) applied to uA and gW = B's own state
    after those same JK steps.  Error ~128^(-JK/2) — far below fp32 noise
    (validated 2e-7 against the exact chain).  This halves the serial chain
    and makes the kernel DMA-paced instead of latency-paced.
    """
    MID, JK = 128, 8
    from contextlib import nullcontext

    nc = bacc.Bacc("TRN2", target_bir_lowering=False, debug=False)
    # host pre-transposes emissions to [row, prev, s, cur] so every partition
    # reads one large contiguous block per chunk (512B descriptors -> 8KB+)
    emits_h = nc.dram_tensor(
        "emits", [RPC, LO, S, LO], mybir.dt.float32, kind="ExternalInput"
    )
    # cols per row r: 3r+0 = uB, 3r+1 = gA, 3r+2 = gW
    alpha_out = nc.dram_tensor(
        "alpha_out", [LO, 3 * RPC], mybir.dt.float32, kind="ExternalOutput"
    )
    e_prsc = emits_h.rearrange("r p s c -> p r s c")

    # Chunk pairs (A-range, B-range) streamed together; the scan steps of all
    # four chains (2 segments x 2 rows) are emitted interleaved per step so
    # the engine FIFOs alternate between chains instead of head-of-line
    # blocking one chain behind another.
    CH = 16
    a_starts = [1] + list(range(CH, MID, CH))
    b_starts = list(range(MID, S, CH))
    pairs = [
        ((a0, min(a0 + CH, MID)), (b0, b0 + CH))
        for a0, b0 in zip(a_starts, b_starts)
    ]

    with tile.TileContext(nc) as tc:
        with (
            tc.tile_pool(name="raw", bufs=6) as raw_pool,
            tc.tile_pool(name="expo", bufs=6) as exp_pool,
            tc.tile_pool(name="keep", bufs=1) as keep_pool,
            tc.tile_pool(name="alpha", bufs=4) as alpha_pool,
            tc.tile_pool(name="init", bufs=1) as init_pool,
            tc.tile_pool(name="outp", bufs=1) as out_pool,
            tc.tile_pool(name="psum", bufs=2, space="PSUM") as psum_pool,
        ):
            bias_t = init_pool.tile([LO, 1], mybir.dt.float32, name="bias_delta")
            nc.vector.memset(bias_t[:, :], -DELTA)

            ones_t = init_pool.tile([LO, 1], MM_DTYPE, name="ones_init")
            nc.vector.memset(ones_t[:, :], 1.0)

            out_tiles = {}

            def step(tag, r, lhsT, out_dt=None):
                """one matvec chain step: alpha[tag,r] <- lhsT^T @ alpha[tag,r]"""
                # the junction chain runs after A finishes; share A's PSUM banks
                # (4 tags x 2 bufs = all 8 banks)
                ptag = "A" if tag == "J" else tag
                ps = psum_pool.tile(
                    [LO, 1], mybir.dt.float32, tag=f"ps{ptag}{r}", name=f"ps_{tag}{r}"
                )
                nc.tensor.matmul(
                    ps[:, :], lhsT, alpha_cur[(tag, r)][:, :], start=True, stop=True
                )
                a_new = alpha_pool.tile(
                    [LO, 1],
                    out_dt or MM_DTYPE,
                    tag=f"al{tag}{r}",
                    name=f"alpha_{tag}{r}",
                )
                nc.vector.tensor_copy(a_new[:, :], ps[:, :])
                alpha_cur[(tag, r)] = a_new

            hw_loop = getattr(_build_program_v2, "_hw_loop", 0)
            loop_ctx = (
                tc.For_i(
                    0,
                    hw_loop,
                    1,
                    hint_engines=(
                        mybir.EngineType.PE,
                        mybir.EngineType.DVE,
                        mybir.EngineType.Activation,
                        mybir.EngineType.SP,
                    ),
                )
                if hw_loop
                else nullcontext()
            )
            with loop_ctx:
              for rep in range(repeats):
                last_rep = rep == repeats - 1
                alpha_cur = {}
                # A chains start from exp(emits[r, 0, 0:128])
                for r in range(RPC):
                    la0 = init_pool.tile(
                        [LO, 1], mybir.dt.float32, name=f"la0_{rep}_{r}"
                    )
                    nc.sync.dma_start(
                        out=la0[:, :],
                        in_=emits_h[r, 0, 0, 0:LO].rearrange("(p one) -> p one", one=1),
                    )
                    a0 = alpha_pool.tile(
                        [LO, 1], MM_DTYPE, tag=f"alA{r}", name=f"alpha0_{r}"
                    )
                    nc.scalar.activation(
                        a0[:, :], la0[:, :], mybir.ActivationFunctionType.Exp
                    )
                    alpha_cur[("A", r)] = a0
                    alpha_cur[("B", r)] = ones_t

                keep_tiles = None
                for pi, ((a0, a1), (b0, b1)) in enumerate(pairs):
                    na, nb = a1 - a0, b1 - b0
                    em = {}
                    for seg, t0, t1, n in (("A", a0, a1, na), ("B", b0, b1, nb)):
                        em_raw = raw_pool.tile(
                            [LO, RPC, n, LO],
                            mybir.dt.float32,
                            tag="raw",
                            name=f"em_raw{seg}",
                        )
                        for r in range(RPC):
                            nc.sync.dma_start(
                                out=em_raw[:, r, :, :], in_=e_prsc[:, r, t0:t1, :]
                            )
                        keep = seg == "B" and pi == 0
                        pool = keep_pool if keep else exp_pool
                        em_exp = pool.tile(
                            [LO, RPC, n, LO],
                            MM_DTYPE,
                            tag="keep" if keep else "expo",
                            name=f"em_exp{seg}",
                        )
                        for r in range(RPC):
                            for g0 in range(0, n, 8):
                                g1 = min(g0 + 8, n)
                                nc.scalar.activation(
                                    em_exp[:, r, g0:g1, :],
                                    em_raw[:, r, g0:g1, :],
                                    mybir.ActivationFunctionType.Exp,
                                    bias=bias_t[:, :],
                                )
                        em[seg] = em_exp
                    if pi == 0:
                        keep_tiles = em["B"]
                    decouple = getattr(_build_program_v2, "_decouple", False)
                    for j in range(max(na, nb)):
                        for seg, n, t0 in (("A", na, a0), ("B", nb, b0)):
                            if j >= n:
                                continue
                            t = t0 + j
                            for r in range(RPC):
                                last_b = seg == "B" and t == S - 1
                                lhs = (
                                    keep_tiles[:, r, j % JK, :]
                                    if (decouple and pi > 0)
                                    else em[seg][:, r, j, :]
                                )
                                step(
                                    seg,
                                    r,
                                    lhs,
                                    out_dt=mybir.dt.float32 if last_b else None,
                                )
                            if seg == "B" and t == MID + JK - 1:
                                # snapshot gW = B state after its first JK steps
                                for r in range(RPC):
                                    gw = out_pool.tile(
                                        [LO, 1], mybir.dt.float32, name=f"gW_{r}"
                                    )
                                    nc.vector.tensor_copy(
                                        gw[:, :], alpha_cur[("B", r)][:, :]
                                    )
                                    out_tiles[("gW", r)] = gw

                if True:
                    # junction: JK steps of B applied to uA (every rep, so the
                    # benchmark repeats carry the same work as the real pass)
                    for r in range(RPC):
                        alpha_cur[("J", r)] = alpha_cur[("A", r)]
                    for j in range(JK):
                        for r in range(RPC):
                            step(
                                "J",
                                r,
                                keep_tiles[:, r, j, :],
                                out_dt=(mybir.dt.float32 if j == JK - 1 else None),
                            )
                    for r in range(RPC):
                        out_tiles[("gA", r)] = alpha_cur[("J", r)]
                        out_tiles[("uB", r)] = alpha_cur[("B", r)]

                if last_rep:
                    for r in range(RPC):
                        for i, name in enumerate(("uB", "gA", "gW")):
                            nc.sync.dma_start(
                                out=alpha_out[:, 3 * r + i : 3 * r + i + 1],
                                in_=out_tiles[(name, r)][:, :],
                            )

    nc.compile()
    return nc


VARIANT = "v2"
BUILDERS_HW = {"v2": _build_program_v2}
_PROGRAM_CACHE = {}


def _builder(repeats=1):
    return (_build_program_v2 if VARIANT == "v2" else _build_program)(repeats)


def _get_program():
    key = VARIANT
    if key not in _PROGRAM_CACHE:
        _PROGRAM_CACHE[key] = _builder()
    return _PROGRAM_CACHE[key]


def kernel(emits, targets, mask):
    global LAST_RESULTS
    emits = np.asarray(emits)
    targets = np.asarray(targets)
    mask = np.asarray(mask)
    assert emits.shape == (B, S, NL) and emits.dtype == np.float32

    # Device-side emissions: exact identity substitution for masked-out steps
    # (graded inputs have mask all ones, so this is normally a no-op view).
    mask_b = mask.astype(bool)
    step_on = mask_b[:, 1:]  # [B, S-1]; step t>=1 applies iff mask[b, t]
    if step_on.all():
        emits_dev = emits
    else:
        emits_dev = emits.copy()
        ident = np.full(NL, -1e30, np.float32)
        ident[np.arange(LO) * LO + np.arange(LO)] = DELTA
        bb, tt = np.nonzero(~step_on)
        emits_dev[bb, tt + 1, :] = ident

    nc = _get_program()
    emits_dev = _prep_emits(emits_dev, nc)
    in_maps = [
        {"emits": np.ascontiguousarray(emits_dev[k * RPC : (k + 1) * RPC])}
        for k in range(N_CORES)
    ]
    res = run_bass_kernel_spmd(nc, in_maps, core_ids=list(range(N_CORES)))
    LAST_RESULTS = res

    # ---- host epilogue (float64)
    n_steps = step_on.sum(axis=1).astype(np.float64)  # unmasked steps per row
    log_z = 0.0
    for k in range(N_CORES):
        alpha = res.results[k]["alpha_out"].astype(np.float64)
        for r in range(RPC):
            b = k * RPC + r
            if VARIANT == "v2":
                uB, gA, gW = (alpha[:, 3 * r + i] for i in range(3))
                log_z += (
                    np.log(uB.sum())
                    + np.log(gA.sum())
                    - np.log(gW.sum())
                    + DELTA * n_steps[b]
                )
            else:
                log_z += np.log(alpha[:, r].sum()) + DELTA * n_steps[b]

    gold = np.take_along_axis(
        emits.reshape(B, S, NL), targets.astype(np.int64)[..., None], axis=-1
    )[..., 0]
    scores = np.where(mask_b, gold.astype(np.float64), 0.0).sum()
    total_token = float(mask_b.sum())
    return np.float32((log_z - scores) / total_token)


def _prep_emits(emits, nc):
    """Reshape the [B?, S, NL] host emissions to whatever per-core layout the
    program's `emits` input declares (handles the [row, prev, s, cur]
    DMA-friendly transpose)."""
    from concourse import mybir as _mybir

    emits = np.asarray(emits, np.float32)
    lead = emits.shape[0]
    for alloc in nc.m.functions[0].allocations:
        if (
            isinstance(alloc, _mybir.MemoryLocationSet)
            and alloc.memorylocations[0].name == "emits"
        ):
            shape = tuple(alloc.tensor_shape)
            break
    else:
        raise KeyError("emits input not found")
    if shape[1:] == (LO, S, LO):  # transposed layout
        return np.ascontiguousarray(
            emits.reshape(lead, S, LO, LO).transpose(0, 2, 1, 3)
        )
    return emits.reshape((lead,) + shape[1:])


def _make_runner(nc, emits):
    """Return a zero-arg callable that runs `nc` once on the 8 cores with
    device-resident inputs (async dispatch; caller blocks on the result).

    Mirrors bass2jax.run_bass_via_pjrt's multi-core path but without output
    donation, so the jitted executable can be re-invoked.
    """
    import jax
    from jax.sharding import Mesh, PartitionSpec, NamedSharding
    from jax.experimental.shard_map import shard_map
    from concourse import bass2jax, mybir as _mybir

    bass2jax.install_neuronx_cc_hook()

    partition_name = nc.partition_id_tensor.name if nc.partition_id_tensor else None
    in_names, out_names, out_avals, zero_outs = [], [], [], []
    for alloc in nc.m.functions[0].allocations:
        if not isinstance(alloc, _mybir.MemoryLocationSet):
            continue
        name = alloc.memorylocations[0].name
        if alloc.kind == "ExternalInput":
            if name != partition_name:
                in_names.append(name)
        elif alloc.kind == "ExternalOutput":
            shape = tuple(alloc.tensor_shape)
            dtype = _mybir.dt.np(alloc.dtype)
            out_names.append(name)
            out_avals.append(jax.core.ShapedArray(shape, dtype))
            zero_outs.append(np.zeros((N_CORES * shape[0], *shape[1:]), dtype))
    assert in_names == ["emits"], in_names
    bind_names = list(in_names) + list(out_names)
    if partition_name is not None:
        bind_names.append(partition_name)

    def _body(*args):
        operands = list(args)
        if partition_name is not None:
            operands.append(bass2jax.partition_id_tensor())
        return tuple(
            bass2jax._bass_exec_p.bind(
                *operands,
                out_avals=tuple(out_avals),
                in_names=tuple(bind_names),
                out_names=tuple(out_names),
                lowering_input_output_aliases=(),
                sim_require_finite=True,
                sim_require_nnan=True,
                nc=nc,
            )
        )

    devices = jax.devices()[:N_CORES]
    mesh = Mesh(np.asarray(devices), ("core",))
    spec = PartitionSpec("core")
    n_args = 1 + len(out_names)
    fn = jax.jit(
        shard_map(
            _body,
            mesh=mesh,
            in_specs=(spec,) * n_args,
            out_specs=(spec,) * len(out_names),
            check_rep=False,
        ),
        keep_unused=True,
    )

    sharding = NamedSharding(mesh, spec)
    emits = _prep_emits(emits, nc)
    emits_dev = jax.device_put(emits, sharding)  # [16,...] -> 2 rows per core
    zeros_dev = [jax.device_put(z, sharding) for z in zero_outs]
    jax.block_until_ready([emits_dev] + zeros_dev)

    def run():
        return fn(emits_dev, *zeros_dev)

    return run


def benchmark(emits, builder=None, loops=(64, 256), rounds=8):
    """Measure on-device kernel time with the hardware-loop slope method:
    build the program with a For_i loop of n_lo and n_hi iterations around
    the body, once with a 1x body and once with a 2x-unrolled body.  The
    double difference
        [ (T(n_hi, 2x) - T(n_lo, 2x)) - (T(n_hi, 1x) - T(n_lo, 1x)) ] / (n_hi - n_lo)
    isolates the marginal per-pass kernel time, cancelling both the multi-ms
    dispatch overhead and the per-iteration loop overhead (back-edge barrier +
    instruction refetch).  Device compute dominates each dispatch, so rounds
    are stable to ~1%."""
    import time

    import jax

    build = builder or BUILDERS_HW[VARIANT]
    n_lo, n_hi = loops
    emits = np.asarray(emits, np.float32).reshape(B, S, NL)

    runners = {}
    for body in (1, 2):
        for n in (n_lo, n_hi):
            build._hw_loop = n
            try:
                runners[(body, n)] = _make_runner(build(repeats=body), emits)
            finally:
                build._hw_loop = 0
    jax.block_until_ready([r() for r in runners.values()])

    med = {}
    obs = {k: [] for k in runners}
    for _ in range(rounds):
        for k, run in runners.items():
            t0 = time.perf_counter()
            jax.block_until_ready(run())
            obs[k].append(time.perf_counter() - t0)
    for k, v in obs.items():
        med[k] = float(np.median(v))
    slope1 = (med[(1, n_hi)] - med[(1, n_lo)]) / (n_hi - n_lo)
    slope2 = (med[(2, n_hi)] - med[(2, n_lo)]) / (n_hi - n_lo)
    kernel_s = slope2 - slope1
    return {
        "per_iter_ns": kernel_s * 1e9,
        "slope1_ns": slope1 * 1e9,
        "loop_overhead_ns": (2 * slope1 - slope2) * 1e9,
        "per_dispatch_ns": med[(1, n_lo)] * 1e9,
    }



# revision 2
# speedup vs baseline: 6.2068x; 6.2068x over previous
# Order-2 CRF loss kernel v3 — Trainium2 (Bass/Tile), 8-core data parallel.
#
# Linear-domain forward chain per batch row (see kernel.py v2 header for the
# derivation): alpha_t = E_t^T alpha_{t-1}, E_t = exp(em_t - DELTA).  v3
# generalizes v2's two-segment split to K=17 parallel segments of L=15 steps
# per row, glued by rank-1 junction corrections (the positive transition
# matrices contract to rank-1 in a few steps):
#   log Z = log sum(u_{K-1}) + sum_k [log sum(g_k) - log sum(w_k)] + 255*DELTA
# where u = last segment's state from ones, w_k = segment k's state after JK
# steps from ones, g_k = those same JK matrices applied to segment k-1's final
# state.  Junction error ~ (contraction)^JK ~ 1e-3 per junction at JK=4,
# far below the fp8/bf16 quantization noise.
#
# Streaming: the host permutes emissions depth-major ([row, prev, j, k, cur],
# fp8 e3m4) so one DMA slice j carries step j of every segment; all 34 chains
# (2 rows x 17 segments) advance one step per slice.  exp is split across
# three engines by segment ranges: ScalarE (exact spline exp), DVE and
# GPSIMD/Pool (Schraudolph bit-trick: bits16 = round(x*A + B) reinterpreted
# as bf16 ~= 2^((x-DELTA)*log2e), calibrated so E[approx/exact] = 1).
# Per-depth matvecs write columns of a shared PSUM tile per row; one DVE copy
# per (row, depth) moves all 17 alphas back to SBUF bf16.

import numpy as np

import concourse.bass as bass  # noqa: F401
import concourse.tile as tile
from concourse import bacc, mybir
from concourse.bass_utils import run_bass_kernel_spmd

B, S, LO = 16, 256, 128
NL = LO * LO
N_CORES = 8
RPC = B // N_CORES  # rows per core = 2
K = 17  # segments per row
L = 15  # steps per segment  (K*L = 255 = S-1)
JK = 2  # junction steps
DELTA = float(np.log(128.0) + 0.5)
LOG2E = float(np.log2(np.e))

# Schraudolph constants for bf16 target, linear-mean calibrated (probe1):
#   bits = x*A_BT + (B_BT0 + B_TUNE)  ->  int16  ->  bitcast bf16
A_BT = 128.0 * LOG2E
B_BT0 = 128.0 * (127.0 - DELTA * LOG2E)
B_TUNE = -7.33

# engine split of the K segments per (row, slice): [ACT, DVE, POOL] shares
N_ACT, N_DVE, N_POOL = 6, 8, 3
assert N_ACT + N_DVE + N_POOL == K

FP8 = mybir.dt.float8e3
MM_DTYPE = mybir.dt.bfloat16

LAST_RESULTS = None


def _build_program(repeats=1):
    from contextlib import nullcontext

    nc = bacc.Bacc("TRN2", target_bir_lowering=False, debug=False)
    emits_h = nc.dram_tensor(
        "emits", [RPC, LO, L, K, LO], FP8, kind="ExternalInput"
    )
    a0_h = nc.dram_tensor("a0", [RPC, LO], mybir.dt.float32, kind="ExternalInput")
    # cols: [u_r0, u_r1 | w_r0 x16, w_r1 x16 | g_r0 x16, g_r1 x16]
    alpha_out = nc.dram_tensor(
        "alpha_out", [LO, 66], mybir.dt.float32, kind="ExternalOutput"
    )
    e_view = emits_h.rearrange("r p j k c -> p r j k c")

    with tile.TileContext(nc) as tc:
        with (
            tc.tile_pool(name="raw", bufs=5) as raw_pool,
            tc.tile_pool(name="expo", bufs=4) as exp_pool,
            tc.tile_pool(name="head", bufs=1) as head_pool,
            tc.tile_pool(name="alpha", bufs=2) as alpha_pool,
            tc.tile_pool(name="init", bufs=1) as init_pool,
            tc.tile_pool(name="outp", bufs=1) as out_pool,
            tc.tile_pool(name="psum", bufs=2, space="PSUM") as psum_pool,
            tc.tile_pool(name="psumj", bufs=1, space="PSUM") as psumj_pool,
        ):
            bias_t = init_pool.tile([LO, 1], mybir.dt.float32, name="bias_delta")
            nc.vector.memset(bias_t[:, :], -DELTA)
            # dummy activation: triggers the exp table-set load (~2.7us)
            # while the first DMA slice is still in flight
            scratch = init_pool.tile([LO, 1], mybir.dt.float32, name="scratch")
            nc.scalar.activation(
                scratch[:, :], bias_t[:, :], mybir.ActivationFunctionType.Exp
            )

            hw_loop = getattr(_build_program, "_hw_loop", 0)
            loop_ctx = (
                tc.For_i(
                    0,
                    hw_loop,
                    1,
                    hint_engines=(
                        mybir.EngineType.PE,
                        mybir.EngineType.DVE,
                        mybir.EngineType.Activation,
                        mybir.EngineType.SP,
                        mybir.EngineType.Pool,
                    ),
                )
                if hw_loop
                else nullcontext()
            )
            with loop_ctx:
              for rep in range(repeats):
                last_rep = rep == repeats - 1
                stage = out_pool.tile(
                    [LO, 66], mybir.dt.float32, name=f"stage_{rep}"
                )

                # ---- fetch plan: singles at both ends (latency-critical),
                # pairs in the middle (SP/HWDGE issue relief).
                groups = (
                    [(0, 1), (1, 2)]
                    + [(j, j + 2) for j in range(2, 12, 2)]
                    + [(12, 13), (13, 14), (14, 15)]
                )
                raw_tiles = {}
                next_group = 0

                def fetch():
                    nonlocal next_group
                    g0, g1 = groups[next_group]
                    next_group += 1
                    raw = raw_pool.tile(
                        [LO, RPC, g1 - g0, K, LO], FP8, tag="raw", name=f"raw_{g0}"
                    )
                    nc.sync.dma_start(
                        out=raw[:, :, :, :, :], in_=e_view[:, :, g0:g1, :, :]
                    )
                    for j in range(g0, g1):
                        raw_tiles[j] = (raw, j - g0)

                fetch()  # slice 0 first so its transfer leads the DMA queue

                # ---- initial alphas (both rows in one [LO, 2K] tile):
                # col r*K = exp(a0_r), the rest ones
                alpha_cur = alpha_pool.tile(
                    [LO, RPC * K], MM_DTYPE, tag="al", name="alpha_init"
                )
                nc.vector.memset(alpha_cur[:, :], 1.0)
                la0 = init_pool.tile([LO, RPC], mybir.dt.float32, name=f"la0_{rep}")
                nc.sync.dma_start(
                    out=la0[:, :],
                    in_=a0_h.rearrange("r p -> p r"),
                )
                fetch()  # slices 1-2

                head_tiles = []  # slices 0..JK-1 exp'd tiles (retained)

                def emit_exp(j):
                    raw, jo = raw_tiles.pop(j)
                    keep = j < JK
                    pool = head_pool if keep else exp_pool
                    ex = pool.tile(
                        [LO, RPC, K, LO],
                        MM_DTYPE,
                        tag=None if keep else "expo",
                        name=f"exp_{j}",
                    )
                    if keep:
                        head_tiles.append(ex)
                    # one cross-row instruction per engine per slice
                    nc.scalar.activation(
                        ex[:, :, 0:N_ACT, :],
                        raw[:, :, jo, 0:N_ACT, :],
                        mybir.ActivationFunctionType.Exp,
                        bias=bias_t[:, :],
                    )
                    nc.vector.tensor_scalar(
                        ex[:, :, N_ACT : N_ACT + N_DVE, :].bitcast(mybir.dt.int16),
                        raw[:, :, jo, N_ACT : N_ACT + N_DVE, :],
                        A_BT,
                        B_BT0 + B_TUNE,
                        mybir.AluOpType.mult,
                        mybir.AluOpType.add,
                    )
                    nc.gpsimd.tensor_scalar(
                        ex[:, :, N_ACT + N_DVE : K, :].bitcast(mybir.dt.int16),
                        raw[:, :, jo, N_ACT + N_DVE : K, :],
                        A_BT,
                        B_BT0 + B_TUNE,
                        mybir.AluOpType.mult,
                        mybir.AluOpType.add,
                    )
                    if next_group < len(groups):
                        fetch()
                    return ex

                # ---- software-pipelined stream: exp for slice j+1 is emitted
                # (and thus FIFO-ordered) BEFORE depth j's matvec batch +
                # alpha copy, so the DVE copy never stalls the exp stream.
                ex_tiles = {0: emit_exp(0)}
                # a0 exps sit after exp_0 in the ACT FIFO (chain col 0 only)
                for r in range(RPC):
                    nc.scalar.activation(
                        alpha_cur[:, r * K : r * K + 1],
                        la0[:, r : r + 1],
                        mybir.ActivationFunctionType.Exp,
                    )

                for j in range(L):
                    if j + 1 < L:
                        ex_tiles[j + 1] = emit_exp(j + 1)
                    ex = ex_tiles.pop(j)

                    ps = psum_pool.tile(
                        [LO, RPC * K], mybir.dt.float32, tag="ps", name=f"ps_{j}"
                    )
                    for r in range(RPC):
                        for k in range(K):
                            c = r * K + k
                            nc.tensor.matmul(
                                ps[:, c : c + 1],
                                ex[:, r, k, :],
                                alpha_cur[:, c : c + 1],
                                start=True,
                                stop=True,
                            )
                    a_new = alpha_pool.tile(
                        [LO, RPC * K], MM_DTYPE, tag="al", name=f"alpha_{j}"
                    )
                    nc.vector.tensor_copy(a_new[:, :], ps[:, :])
                    alpha_cur = a_new
                    if j == JK - 1 and last_rep:
                        # w_k snapshot (fp32, straight from PSUM):
                        # ps cols r*K+1 .. r*K+16 -> stage cols 2..34
                        nc.vector.tensor_copy(
                            stage[:, 2:34],
                            ps[:, :].rearrange("p (r k) -> p r k", r=RPC)[:, :, 1:K],
                        )
                    if j == L - 1 and last_rep:
                        # u = last segment's final state (ps col r*K+16)
                        nc.vector.tensor_copy(
                            stage[:, 0:2],
                            ps[:, :].rearrange("p (r k) -> p r k", r=RPC)[
                                :, :, K - 1 : K
                            ],
                        )
                        # u+w output can ship now, overlapping the junction
                        nc.sync.dma_start(
                            out=alpha_out[:, 0:34], in_=stage[:, 0:34]
                        )

                # ---- junctions: JK steps of segment k's head applied to
                # segment k-1's final state, for k = 1..16; both rows share
                # one PSUM tile (cols r*16 + k-1)
                KJ = K - 1
                alpha_j = None
                for d in range(JK):
                    psj = psumj_pool.tile(
                        [LO, RPC * KJ], mybir.dt.float32, tag="psj", name=f"psj_{d}"
                    )
                    for r in range(RPC):
                        for k in range(1, K):
                            src = (
                                alpha_j[:, r * KJ + k - 1 : r * KJ + k]
                                if d > 0
                                else alpha_cur[:, r * K + k - 1 : r * K + k]
                            )
                            nc.tensor.matmul(
                                psj[:, r * KJ + k - 1 : r * KJ + k],
                                head_tiles[d][:, r, k, :],
                                src,
                                start=True,
                                stop=True,
                            )
                    if d < JK - 1:
                        aj = alpha_pool.tile(
                            [LO, RPC * KJ], MM_DTYPE, tag="aj", name=f"alphaj_{d}"
                        )
                        nc.vector.tensor_copy(aj[:, :], psj[:, :])
                        alpha_j = aj
                    elif last_rep:
                        # g_k final (fp32 from PSUM)
                        nc.vector.tensor_copy(stage[:, 34:66], psj[:, :])
                        nc.sync.dma_start(
                            out=alpha_out[:, 34:66], in_=stage[:, 34:66]
                        )

    nc.compile()
    return nc


_PROGRAM_CACHE = {}


def _get_program():
    if "v3" not in _PROGRAM_CACHE:
        _PROGRAM_CACHE["v3"] = _build_program()
    return _PROGRAM_CACHE["v3"]


def _prep_inputs(emits, mask_b=None):
    """Permute [B,S,NL] fp32 emissions to the depth-major per-core device
    layout: emits8 [B, 128(prev), L, K, 128(cur)] fp8e3 + a0 [B, 128] fp32."""
    emits = np.asarray(emits, np.float32)
    em = emits[:, 1:, :].reshape(B, K, L, LO, LO)  # [b, k, j, p, c]
    if mask_b is not None:
        step_on = mask_b[:, 1:]  # [B, S-1]
        if not step_on.all():
            em = em.copy()
            ident = np.full((LO, LO), -20.0, np.float32)
            np.fill_diagonal(ident, DELTA)
            bb, ss = np.nonzero(~step_on)
            kk, jj = ss // L, ss % L
            em[bb, kk, jj] = ident
    em8 = np.ascontiguousarray(em.transpose(0, 3, 2, 1, 4)).astype(
        mybir.dt.np(FP8)
    )  # [b, p, j, k, c]
    a0 = np.ascontiguousarray(emits[:, 0, 0:LO])  # BOS=0 -> first 128 entries
    return em8, a0


def kernel(emits, targets, mask):
    global LAST_RESULTS
    emits = np.asarray(emits)
    targets = np.asarray(targets)
    mask_b = np.asarray(mask).astype(bool)
    assert emits.shape == (B, S, NL) and emits.dtype == np.float32

    em8, a0 = _prep_inputs(emits, mask_b)
    nc = _get_program()
    in_maps = [
        {
            "emits": np.ascontiguousarray(em8[k * RPC : (k + 1) * RPC]),
            "a0": np.ascontiguousarray(a0[k * RPC : (k + 1) * RPC]),
        }
        for k in range(N_CORES)
    ]
    res = run_bass_kernel_spmd(nc, in_maps, core_ids=list(range(N_CORES)))
    LAST_RESULTS = res

    # ---- host epilogue (float64)
    n_steps = mask_b[:, 1:].sum(axis=1).astype(np.float64)
    log_z = 0.0
    for c in range(N_CORES):
        alpha = res.results[c]["alpha_out"].astype(np.float64)
        for r in range(RPC):
            b = c * RPC + r
            u = alpha[:, r]
            w = alpha[:, 2 + 16 * r : 18 + 16 * r]
            g = alpha[:, 34 + 16 * r : 50 + 16 * r]
            log_z += (
                np.log(u.sum())
                + np.log(g.sum(axis=0)).sum()
                - np.log(w.sum(axis=0)).sum()
                + DELTA * n_steps[b]
            )

    gold = np.take_along_axis(
        emits.reshape(B, S, NL), targets.astype(np.int64)[..., None], axis=-1
    )[..., 0]
    scores = np.where(mask_b, gold.astype(np.float64), 0.0).sum()
    total_token = float(mask_b.sum())
    return np.float32((log_z - scores) / total_token)


def _make_runner(nc, emits):
    """Zero-arg callable running `nc` once on 8 cores with device-resident
    inputs (async dispatch; caller blocks on result)."""
    import jax
    from jax.sharding import Mesh, NamedSharding, PartitionSpec
    from jax.experimental.shard_map import shard_map
    from concourse import bass2jax, mybir as _mybir

    bass2jax.install_neuronx_cc_hook()

    em8, a0 = _prep_inputs(np.asarray(emits, np.float32).reshape(B, S, NL))
    host_inputs = {"emits": em8, "a0": a0}

    partition_name = nc.partition_id_tensor.name if nc.partition_id_tensor else None
    in_names, out_names, out_avals, zero_outs = [], [], [], []
    for alloc in nc.m.functions[0].allocations:
        if not isinstance(alloc, _mybir.MemoryLocationSet):
            continue
        name = alloc.memorylocations[0].name
        if alloc.kind == "ExternalInput":
            if name != partition_name:
                in_names.append(name)
        elif alloc.kind == "ExternalOutput":
            shape = tuple(alloc.tensor_shape)
            dtype = _mybir.dt.np(alloc.dtype)
            out_names.append(name)
            out_avals.append(jax.core.ShapedArray(shape, dtype))
            zero_outs.append(np.zeros((N_CORES * shape[0], *shape[1:]), dtype))
    bind_names = list(in_names) + list(out_names)
    if partition_name is not None:
        bind_names.append(partition_name)

    def _body(*args):
        operands = list(args)
        if partition_name is not None:
            operands.append(bass2jax.partition_id_tensor())
        return tuple(
            bass2jax._bass_exec_p.bind(
                *operands,
                out_avals=tuple(out_avals),
                in_names=tuple(bind_names),
                out_names=tuple(out_names),
                lowering_input_output_aliases=(),
                sim_require_finite=True,
                sim_require_nnan=True,
                nc=nc,
            )
        )

    devices = jax.devices()[:N_CORES]
    mesh = Mesh(np.asarray(devices), ("core",))
    spec = PartitionSpec("core")
    n_args = len(in_names) + len(out_names)
    fn = jax.jit(
        shard_map(
            _body,
            mesh=mesh,
            in_specs=(spec,) * n_args,
            out_specs=(spec,) * len(out_names),
            check_rep=False,
        ),
        keep_unused=True,
    )

    sharding = NamedSharding(mesh, spec)
    ins_dev = [jax.device_put(host_inputs[n], sharding) for n in in_names]
    zeros_dev = [jax.device_put(z, sharding) for z in zero_outs]
    jax.block_until_ready(ins_dev + zeros_dev)

    def run():
        return fn(*ins_dev, *zeros_dev)

    return run


def benchmark(emits, builder=None, loops=(64, 256), rounds=8):
    """On-device kernel time via the hardware-loop slope method (see v2)."""
    import time

    import jax

    build = builder or _build_program
    n_lo, n_hi = loops
    emits = np.asarray(emits, np.float32).reshape(B, S, NL)

    runners = {}
    for body in (1, 2):
        for n in (n_lo, n_hi):
            build._hw_loop = n
            try:
                runners[(body, n)] = _make_runner(build(repeats=body), emits)
            finally:
                build._hw_loop = 0
    jax.block_until_ready([r() for r in runners.values()])

    med = {}
    obs = {k: [] for k in runners}
    for _ in range(rounds):
        for kk, run in runners.items():
            t0 = time.perf_counter()
            jax.block_until_ready(run())
            obs[kk].append(time.perf_counter() - t0)
    for kk, v in obs.items():
        med[kk] = float(np.median(v))
    slope1 = (med[(1, n_hi)] - med[(1, n_lo)]) / (n_hi - n_lo)
    slope2 = (med[(2, n_hi)] - med[(2, n_lo)]) / (n_hi - n_lo)
    kernel_s = slope2 - slope1
    return {
        "per_iter_ns": kernel_s * 1e9,
        "slope1_ns": slope1 * 1e9,
        "loop_overhead_ns": (2 * slope1 - slope2) * 1e9,
        "per_dispatch_ns": med[(1, n_lo)] * 1e9,
    }
